# revision 29
# baseline (speedup 1.0000x reference)
"""Trainium2 Bass kernel: multi-head attention forward (B=2, S=2048, D=1024, H=16).

Sharding: 8 cores = data-parallel over batch (2) x tensor-parallel over heads
(4 head-groups of 4 heads).  Host pre-transposes/slices per core; the 4
partial outputs per batch are summed (f32) and bias added on the host.

v4 design (fp8e4 DoubleRow projections with residual compensation, bf16
attention core):
  - x and W{q,k,v} are pre-scaled by powers of 2 (x*8, W*256) and shipped as
    fp8e4 MAIN + RESIDUAL pairs (residual = fp8(a - fp8(a)) lands in the
    subnormal range of the same scale, so main+residual restores ~bf16
    accuracy); projections run as DoubleRow fp8 matmuls (two 128-deep
    contraction k-tiles per instruction at 0.5 PE cycles/out-col):
    q/k = x1@(W1+W2) (2x fewer PE cycles than bf16), v = x1@(W1+W2)+x2@W1
    (the x-residual term matters for v accuracy; q/k tolerate skipping it)
  - attention stays bf16 end-to-end: fp8 STORAGE of attn/v/ctx was measured
    to cost 2-3e-2 absmax relerr each (kills the 2e-2 gate), and scores are
    PSUM-write-bound so fp8 would not speed them up
  - projections qT/kT (weights stationary) and v (x stationary), bf16 stores
  - scores per (head, 1024-col query block, sk-tile): kT stationary,
    out [sk, q] strip in PSUM; exp on ACT -> bf16 attn strip (SBUF);
    causal diagonal masked by a DVE multiply with a precomputed
    lower-triangle bf16 tile (4x all-SBUF mode, no Q7 launch latency)
  - TRANSPOSED PV: attn strip 128-col slice stationary, v[:, ski, h] (65 cols
    incl. the ones column) moving -> ctx accumulators [q, 65] per query tile;
    free dim 65 instead of W (2x fewer PE cycles than the [65, W] orientation)
  - softmax denominators land as a per-partition COLUMN (acc col 64):
    batched fast-reciprocal [128, QPB] + per-qt tensor_scalar multiply
  - ctx[q, dh] for head pairs re-transposed to ctxT[dh, s] by PE-transpose
    against a bf16 identity (exact), then the row-sharded Wo projection
  - q/k/v projection chains and output-projection tiles are woven into the
    ACT-bound attention blocks as PE filler work (emission order = priority)
"""

import sys

sys.path.insert(0, "/opt/trn_rl_repo")

import numpy as np
import ml_dtypes

BF16 = ml_dtypes.bfloat16
FP8 = ml_dtypes.float8_e4m3  # TRN fp8e4: e4m3, max normal 240

B, S, D = 2, 2048, 1024
H = 16
DH = 64
HL = 4  # heads per core
NCORES = 8

_PROGRAM_CACHE = {}


def build_program(S=S, D=D, HL=HL, DH=DH):
    import concourse.tile as tile
    from concourse import bacc, mybir

    f32 = mybir.dt.float32
    bf = mybir.dt.bfloat16
    f8 = mybir.dt.float8e4
    A = mybir.ActivationFunctionType
    Alu = mybir.AluOpType
    DR = mybir.MatmulPerfMode.DoubleRow

    KD = D // 128          # contraction chunks for the projections (8)
    M = HL * DH            # per-core projected width (256)
    MQ = M // 128          # qT/kT partition tiles (2)
    ST = S // 128          # 128-row s tiles (16)
    NB = S // 512          # 512-col blocks of s (4)
    W = min(1024, S)       # query-block width
    NJ = S // W            # query blocks (2)
    QPB = W // 128         # query tiles per block (8)
    CW = W // 512          # 512-chunks per query block (2)
    NG = (QPB + 3) // 4    # psum bank groups for ctx accumulators

    nc = bacc.Bacc("TRN2", target_bir_lowering=False, debug=False)
    # x arrives x8 in fp8 main (xT) + residual (xT2); W{q,k,v} arrive x256
    # as fp8 [D, 2, M] main/residual stacks.  Wo stays bf16.
    xT = nc.dram_tensor("xT", (D, S), f8, kind="ExternalInput").ap()
    xT2 = nc.dram_tensor("xT2", (D, S), f8, kind="ExternalInput").ap()
    wq = nc.dram_tensor("wq", (D, 2, M), f8, kind="ExternalInput").ap()
    wk = nc.dram_tensor("wk", (D, 2, M), f8, kind="ExternalInput").ap()
    wv = nc.dram_tensor("wv", (D, 2, M), f8, kind="ExternalInput").ap()
    wo = nc.dram_tensor("wo", (M, D), bf, kind="ExternalInput").ap()
    out = nc.dram_tensor("out", (S, D), bf, kind="ExternalOutput").ap()

    with tile.TileContext(nc) as tc:
        with (
            tc.tile_pool(name="wx", bufs=1) as wpool,
            tc.tile_pool(name="persist", bufs=1) as mpool,
            tc.tile_pool(name="attn", bufs=9) as apool,
            tc.tile_pool(name="ctxn", bufs=3) as npool,
            tc.tile_pool(name="rp", bufs=3) as rpool,
            tc.tile_pool(name="ost", bufs=5) as opool,
            tc.tile_pool(name="mm", bufs=2, space="PSUM") as mm,
            tc.tile_pool(name="sc", bufs=2, space="PSUM") as spool,
            tc.tile_pool(name="cx", bufs=1, space="PSUM") as cpool,
        ):
            # ---------------- persistent SBUF tiles -----------------------
            wq_sb = wpool.tile([128, KD, 2, M], f8, tag="wq")
            wk_sb = wpool.tile([128, KD, 2, M], f8, tag="wk")
            wv_sb = wpool.tile([128, KD, 2, M], f8, tag="wv")
            wo_sb = wpool.tile([128, MQ, D], bf, tag="wo")
            xt = wpool.tile([128, KD, S], f8, tag="xt")
            xt2 = wpool.tile([128, KD, S], f8, tag="xt2")
            id_sb = wpool.tile([128, 128], bf, tag="id")
            lt_sb = wpool.tile([128, 128], bf, tag="lt")
            qT_sb = mpool.tile([128, MQ, S], bf, tag="qT")
            kT_sb = mpool.tile([128, MQ, S], bf, tag="kT")
            v_sb = mpool.tile([128, ST, HL * (DH + 1)], bf, tag="v")
            ctxT_sb = mpool.tile([128, MQ, S], bf, tag="ctxT")

            # ---------------- input DMA (interleaved) ---------------------
            wq_r = wq.rearrange("(k p) t m -> p k t m", p=128)
            wk_r = wk.rearrange("(k p) t m -> p k t m", p=128)
            wv_r = wv.rearrange("(k p) t m -> p k t m", p=128)
            xT_r = xT.rearrange("(k p) s -> p k s", p=128)
            xT2_r = xT2.rearrange("(k p) s -> p k s", p=128)

            # consolidated DMAs (HWDGE issue and the DMA device are serial,
            # and every DMA instruction costs ~1us of DGE setup): few, big
            # transfers in need-order — wq/wk + the first x half feed the
            # first scores chains, the rest streams in behind them.
            xq = min(512, W)
            nc.sync.dma_start(wq_sb[:], wq_r[:])
            nc.sync.dma_start(xt[:, :, 0:xq], xT_r[:, :, 0:xq])
            nc.sync.dma_start(wk_sb[:], wk_r[:])
            if W > xq:
                nc.sync.dma_start(xt[:, :, xq:W], xT_r[:, :, xq:W])
            nc.sync.dma_start(wv_sb[:], wv_r[:])
            nc.sync.dma_start(xt2[:, :, 0:W], xT2_r[:, :, 0:W])
            if S > W:
                # x's second half feeds the j1 qk chains (block 3 of the
                # interleave, ~20us in) — it must beat wo, which is only
                # needed by the output projection much later
                nc.sync.dma_start(xt[:, :, W:S], xT_r[:, :, W:S])
                nc.sync.dma_start(xt2[:, :, W:S], xT2_r[:, :, W:S])
            nc.sync.dma_start(wo_sb[:], wo.rearrange("(k p) d -> p k d", p=128))
            # bf16 identity for PE transposes (exact: one nonzero per column)
            junk_sb = wpool.tile([128, 512], bf, tag="junk")
            nc.gpsimd.memset(junk_sb[:], 0.0)
            nc.gpsimd.memset(id_sb[:], 1.0)
            nc.gpsimd.memset(lt_sb[:], 1.0)
            nc.gpsimd.affine_select(
                out=id_sb[:],
                in_=id_sb[:],
                compare_op=Alu.is_equal,
                fill=0.0,
                base=0,
                pattern=[[1, 128]],
                channel_multiplier=-1,
            )
            # causal mask tile: lt[p, f] = 1 where f >= p else 0
            nc.gpsimd.affine_select(
                out=lt_sb[:],
                in_=lt_sb[:],
                compare_op=Alu.is_ge,
                fill=0.0,
                base=0,
                pattern=[[1, 128]],
                channel_multiplier=-1,
            )

            # PE p-state warmup: the tensor engine is at 0.65-1.2 GHz until
            # ~3us of continuous work; burn junk matmuls during the input-DMA
            # window so the first real chains run at the full 2.4 GHz
            wu = mm.tile([128, 512], f32, tag="mm", name="warmup")
            for _ in range(14):
                nc.tensor.matmul(
                    wu[:], id_sb[:], junk_sb[:], start=True, stop=True
                )
            # preload the Exp activation table during the DMA window so the
            # first real exp doesn't pay the 1.3us table load
            warm_sb = wpool.tile([128, 1], bf, tag="warm")
            nc.scalar.activation(warm_sb[:], junk_sb[:, 0:1], A.Exp)

            # ---------------- projection chain units ----------------------
            # DoubleRow fp8: stationary [128, 2, 128] = two 128-deep
            # contraction k-tiles per instruction at 0.5 cycles/out-col;
            # moving free is capped at 512, so 256-col chunks.  Main and
            # W-residual terms accumulate into the same psum columns.
            def emit_qk_chain(which, m, n):
                w_sb, dst = (wq_sb, qT_sb) if which == "q" else (wk_sb, kT_sb)
                sl = slice(512 * n, 512 * (n + 1))
                msl = slice(m * 128, (m + 1) * 128)
                ps = mm.tile([128, 512], f32, tag="mm", name="psqk")
                for c2 in range(2):
                    csl = slice(512 * n + 256 * c2, 512 * n + 256 * c2 + 256)
                    osl = slice(256 * c2, 256 * c2 + 256)
                    for t in range(2):        # W main, then W residual
                        for kp in range(KD // 2):
                            nc.tensor.matmul(
                                ps[:, osl],
                                w_sb[:, 2 * kp:2 * kp + 2, t, msl],
                                xt[:, 2 * kp:2 * kp + 2, csl],
                                start=(c2 == 0 and t == 0 and kp == 0),
                                stop=(c2 == 1 and t == 1 and kp == KD // 2 - 1),
                                perf_mode=DR,
                            )
                # qT stored = 8*q in bf16 (psum = 2048*q)
                nc.vector.tensor_scalar(
                    dst[:, m, sl], ps[:], 2.0 ** -8, None, Alu.mult
                )

            def emit_v_chain(st):
                ssl = slice(st * 128, (st + 1) * 128)
                ps = mm.tile([128, M], f32, tag="mm", name="psv")
                # x1@(W1+W2) + x2@W1: the x-residual term matters for v
                for t in range(2):
                    for kp in range(KD // 2):
                        nc.tensor.matmul(
                            ps[:],
                            xt[:, 2 * kp:2 * kp + 2, ssl],
                            wv_sb[:, 2 * kp:2 * kp + 2, t, :],
                            start=(t == 0 and kp == 0),
                            stop=False,
                            perf_mode=DR,
                        )
                for kp in range(KD // 2):
                    nc.tensor.matmul(
                        ps[:],
                        xt2[:, 2 * kp:2 * kp + 2, ssl],
                        wv_sb[:, 2 * kp:2 * kp + 2, 0, :],
                        start=False,
                        stop=(kp == KD // 2 - 1),
                        perf_mode=DR,
                    )
                vg = v_sb[:, st].rearrange("p (h c) -> p h c", h=HL)
                # v stored at true scale in bf16 (psum = 2048*v)
                nc.vector.tensor_scalar(
                    vg[:, :, 0:DH],
                    ps[:].rearrange("p (h c) -> p h c", h=HL),
                    2.0 ** -11,
                    None,
                    Alu.mult,
                )
                # ones column for the PV denominator trick
                nc.gpsimd.memset(vg[:, :, DH], 1.0)

            # ---------------- output projection unit ----------------------
            def emit_outproj_st(st, tail=False):
                o_sb = opool.tile([128, D], bf, tag="o", name="o_sb")
                if tail and D == 1024 and W == 1024:
                    # attention is over: the scores-pool banks are free, so
                    # compute both 512-chunks into one 2-bank tile and stage
                    # with a single copy, alternating DVE/ACT (both idle)
                    ops = spool.tile([128, D], f32, tag="sc", name="opwide")
                    for n2 in range(D // 512):
                        for p2 in range(MQ):
                            nc.tensor.matmul(
                                ops[:, n2 * 512:(n2 + 1) * 512],
                                ctxT_sb[:, p2, st * 128:(st + 1) * 128],
                                wo_sb[:, p2, n2 * 512:(n2 + 1) * 512],
                                start=(p2 == 0),
                                stop=(p2 == MQ - 1),
                            )
                    # stage off the critical engines: Pool is idle, DVE is
                    # loaded, ACT is the exp bottleneck (never use it here)
                    if st % 2 == 1:
                        nc.gpsimd.tensor_copy(o_sb[:], ops[:])
                    else:
                        nc.vector.tensor_copy(o_sb[:], ops[:])
                else:
                    for n2 in range(D // 512):
                        ops = mm.tile([128, 512], f32, tag="mm", name="opch")
                        for p2 in range(MQ):
                            nc.tensor.matmul(
                                ops[:],
                                ctxT_sb[:, p2, st * 128:(st + 1) * 128],
                                wo_sb[:, p2, n2 * 512:(n2 + 1) * 512],
                                start=(p2 == 0),
                                stop=(p2 == MQ - 1),
                            )
                        if n2 % 2 == 1:
                            nc.gpsimd.tensor_copy(
                                o_sb[:, n2 * 512:(n2 + 1) * 512], ops[:]
                            )
                        else:
                            nc.vector.tensor_copy(
                                o_sb[:, n2 * 512:(n2 + 1) * 512], ops[:]
                            )
                nc.sync.dma_start(out[st * 128:(st + 1) * 128, :], o_sb[:])

            # ---------------- attention block (h, j) ----------------------
            ctxn_tiles = {}

            def attention_hj(h, j, companion=None, lag=7):
                hm, po = h // 2, 64 * (h % 2)
                qrow = slice(po, po + DH)
                nski = QPB * (j + 1)
                accs = [cpool.tile([128, 512], f32, tag=f"cx{g}", name=f"acc{g}")
                        for g in range(NG)]
                if h % 2 == 0:
                    ctxn_tiles[hm] = npool.tile(
                        [128, QPB, 128], bf, tag="cn", name="ctxn"
                    )
                ctx_n = ctxn_tiles[hm]
                pending = []

                def emit_norm_bank(g):
                    # reciprocal of the ones-column sums (col 64 of each acc
                    # slot), then per-qt per-partition scale into ctx_n.
                    # NOTE custom-DVE ops must read SBUF: stage dens there.
                    nqt = min(4, QPB - 4 * g)
                    dens = rpool.tile([128, 4], f32, tag="dens", name="dens")
                    rcp = rpool.tile([128, 4], f32, tag="rcp", name="rcp")
                    nc.vector.tensor_copy(
                        dens[:, 0:nqt],
                        accs[g][:, 0:65 * nqt].rearrange(
                            "p (q c) -> p q c", c=65
                        )[:, :, 64],
                    )
                    nc.vector.reciprocal_approx_fast(
                        out=rcp[:, 0:nqt], in_=dens[:, 0:nqt]
                    )
                    for qi in range(nqt):
                        qt = 4 * g + qi
                        nc.vector.tensor_scalar(
                            ctx_n[:, qt, po:po + DH],
                            accs[g][:, 65 * qi:65 * qi + DH],
                            rcp[:, qi:qi + 1],
                            None,
                            Alu.mult,
                        )
                        if h % 2 == 1:
                            # head pair done for this query tile: transpose
                            # to ctxT right away so the output projection of
                            # this s-tile unlocks without waiting on the
                            # whole block
                            tp = mm.tile([128, 128], bf, tag="mm", name="tp")
                            with tc.high_priority(offset=1_000_000):
                                nc.tensor.transpose(
                                    tp[:], ctx_n[:, qt, :], id_sb[:]
                                )
                            nc.vector.tensor_copy(
                                ctxT_sb[:, hm, (j * QPB + qt) * 128:
                                        (j * QPB + qt + 1) * 128],
                                tp[:],
                            )

                def emit_pv(item):
                    with tc.high_priority(offset=1_000_000):
                        emit_pv_inner(item)

                def emit_pv_inner(item):
                    # one accumulation group per PSUM bank (start zeroes the
                    # whole 2KB zero-region): start on the bank's first PV
                    # (ski=0, qt%4==0), stop on its last (qt%4==3 diagonal)
                    ski, attn_t, x0r = item
                    for qt in range(x0r // 128, QPB):
                        qg = j * QPB + qt
                        last = ski == qg and (qt % 4 == 3 or qt == QPB - 1)
                        nc.tensor.matmul(
                            accs[qt // 4][:, 65 * (qt % 4):65 * (qt % 4) + 65],
                            attn_t[:, qt * 128:(qt + 1) * 128],
                            v_sb[:, ski, h * (DH + 1):(h + 1) * (DH + 1)],
                            start=(ski == 0 and qt % 4 == 0),
                            stop=last,
                        )
                        if last:
                            emit_norm_bank(qt // 4)

                for ski in range(nski):
                    x0r = max(0, 128 * ski - j * W)
                    sc = spool.tile([128, W], f32, tag="sc", name="sc")
                    # scores feed the ACT-critical exp stream: outrank the
                    # PE filler work (priority only reorders ready work,
                    # dependencies still follow emission order)
                    with tc.high_priority(offset=1_000_000):
                        for c in reversed(range(CW)):
                            lo = max(x0r, 512 * c)
                            hi = 512 * (c + 1)
                            if lo >= hi:
                                continue
                            nc.tensor.matmul(
                                sc[:, lo:hi],
                                kT_sb[qrow, hm, ski * 128:(ski + 1) * 128],
                                qT_sb[qrow, hm, j * W + lo:j * W + hi],
                                start=True,
                                stop=True,
                            )
                    attn_t = apool.tile([128, W], bf, tag="attn", name="attn")
                    # scores psum = (8q)(8k) = 64*score; softmax scale 1/8
                    nc.scalar.activation(
                        attn_t[:, x0r:W], sc[:, x0r:W], A.Exp, scale=2.0 ** -9
                    )
                    if ski >= QPB * j:  # diagonal tile: zero strictly-future q
                        # elementwise multiply with the precomputed causal
                        # tile on DVE (all-SBUF bf16 = 4x mode, no Q7 launch)
                        with tc.high_priority(offset=1_000_000):
                            nc.vector.tensor_tensor(
                                attn_t[:, x0r:x0r + 128],
                                attn_t[:, x0r:x0r + 128],
                                lt_sb[:],
                                mybir.AluOpType.mult,
                            )
                    pending.append((ski, attn_t, x0r))
                    if len(pending) >= lag:
                        emit_pv(pending.pop(0))
                    if companion is not None:
                        companion()
                for item in pending:
                    emit_pv(item)


            # ---------------- driver: weave fillers into attention --------
            from collections import deque

            fillers = deque()

            def companion():
                if fillers:
                    fillers.popleft()()

            if NJ == 1:
                # small-S fallback (sim sizes): serial prologue
                for m in range(MQ):
                    for n in range(NB):
                        emit_qk_chain("q", m, n)
                for m in range(MQ):
                    for n in range(NB):
                        emit_qk_chain("k", m, n)
                for st in range(ST):
                    emit_v_chain(st)
                for h in range(HL):
                    attention_hj(h, 0)
                for st in range(ST):
                    emit_outproj_st(st)
            else:
                # NJ == 2: interleave j0 (ACT-light) and j1 (ACT-heavy)
                # blocks per head pair so neither engine starves, and so
                # ctxT tiles complete early enough to overlap the output
                # projection with late attention.
                qk = emit_qk_chain
                blocks = [(0, 0), (1, 0), (0, 1), (1, 1),
                          (2, 0), (3, 0), (2, 1), (3, 1)]
                # all projection chains emitted upfront in need-order;
                # attention-critical PE ops are priority-boosted, so chains
                # run as gap fillers wherever the PE has slack
                emit_qk_chain("q", 0, 0)
                emit_qk_chain("q", 0, 1)
                emit_qk_chain("k", 0, 0)
                emit_v_chain(0)
                qk("k", 0, 1)
                for st in range(1, QPB):
                    emit_v_chain(st)
                for n in (2, 3):
                    qk("q", 0, n)
                for n in (2, 3):
                    qk("k", 0, n)
                for st in range(QPB, 2 * QPB):
                    emit_v_chain(st)
                for n in (0, 1):
                    qk("q", 1, n)
                for n in (0, 1):
                    qk("k", 1, n)
                for n in (2, 3):
                    qk("q", 1, n)
                for n in (2, 3):
                    qk("k", 1, n)
                pre = {
                    6: [lambda st=st: emit_outproj_st(st)
                           for st in range(QPB)],
                }
                for bi, (h, j) in enumerate(blocks):
                    for f in pre.get(bi, ()):
                        fillers.append(f)
                    attention_hj(h, j, companion=companion,
                                 lag=3 if bi == len(blocks) - 1 else 7)
                while fillers:
                    fillers.popleft()()
                for st in range(QPB, 2 * QPB):
                    emit_outproj_st(st, tail=True)

    nc.compile()
    return nc


def _get_program():
    key = (S, D, HL, DH)
    if key not in _PROGRAM_CACHE:
        _PROGRAM_CACHE[key] = build_program(*key)
    return _PROGRAM_CACHE[key]


def _f8_pair(a):
    """main = fp8(a), residual = fp8(a - main) stacked on axis 1."""
    a = np.ascontiguousarray(a, dtype=np.float32)
    m = a.astype(FP8)
    r = (a - m.astype(np.float32)).astype(FP8)
    return np.ascontiguousarray(np.stack([m, r], axis=1))


def make_in_maps(x, Wq, Wk, Wv, Wo):
    x = np.asarray(x, dtype=np.float32)
    xT8 = [np.ascontiguousarray(x[b].T) * 8.0 for b in range(B)]
    xTs = [a.astype(FP8) for a in xT8]
    xT2s = [(a - m.astype(np.float32)).astype(FP8) for a, m in zip(xT8, xTs)]
    Wq = np.asarray(Wq, dtype=np.float32)
    Wk = np.asarray(Wk, dtype=np.float32)
    Wv = np.asarray(Wv, dtype=np.float32)
    Wo = np.asarray(Wo, dtype=np.float32)
    in_maps = []
    for c in range(NCORES):
        b, g = divmod(c, NCORES // B)
        sl = slice(HL * DH * g, HL * DH * (g + 1))
        in_maps.append(
            {
                "xT": xTs[b],
                "xT2": xT2s[b],
                "wq": _f8_pair(Wq[sl, :].T * 256.0),
                "wk": _f8_pair(Wk[sl, :].T * 256.0),
                "wv": _f8_pair(Wv[sl, :].T * 256.0),
                "wo": np.ascontiguousarray(Wo[:, sl].T).astype(BF16),
            }
        )
    return in_maps


def kernel(x, Wq, Wk, Wv, Wo, bo):
    from concourse import bass2jax

    nc = _get_program()
    in_maps = make_in_maps(x, Wq, Wk, Wv, Wo)
    try:
        res = bass2jax.run_bass_via_pjrt(nc, in_maps, n_cores=NCORES)
    except Exception:
        # a wedged neuron core from a prior process fails the first
        # attempt and recovers on retry
        res = bass2jax.run_bass_via_pjrt(nc, in_maps, n_cores=NCORES)
    outs = [np.asarray(res[c]["out"]).astype(np.float32) for c in range(NCORES)]
    gpb = NCORES // B
    o = np.stack([sum(outs[b * gpb + g] for g in range(gpb)) for b in range(B)])
    o = o + np.asarray(bo, dtype=np.float32)[None, None, :]
    return o.astype(np.float32)

